# revision 38
# baseline (speedup 1.0000x reference)
"""BagModel (segment_reduce) Trainium2 kernel.

Computes out = (1/64 * segment_sum(relu(x @ W1 + b1))) @ W2 + b2 for
4096 bags of exactly 64 consecutive rows each, sharded bag-aligned
across 8 NeuronCores (512 bags / 32768 rows per core, weights
replicated, no cross-core communication).

Layout trick: the host permutes each core's x-shard so row-group g
contains row g of every bag, with the contraction dim D on partitions
(xh[p, sg, k, j, b] = x[b*64 + 2*sg + j, 128*k + p]).  The per-bag
segment-sum falls out of PSUM matmul accumulation: the second (W2)
matmul accumulates over the 64 row-groups with start/stop flags, so no
explicit reduction pass over h is ever needed.  The 4 H-slices of the
W2 matmul go to 4 distinct PE column-groups (tile_position) and
genuinely overlap in the array (~50ns/quad issue spacing inside a
batched burst); all four accumulate into ONE shared PSUM bank on
disjoint partitions (32m .. 32m+10).

fp8: x and W1 are quantized to float8e4 (e4m3) on the host and the W1
matmul runs in MatmulPerfMode.DoubleRow, packing the full D=256
contraction into one instruction per (group, h-slice) at 2x bf16
throughput (measured 213ns per [128,2,512]-in/[128,512]-out matmul).
End-to-end rel err vs the fp32 reference is 1.373e-2 (fp8 matmul
quantization dominates; verified identical in host simulation), within
the 2e-2 gate.  htr/W2 stay fp16 (fp8 htr would land at ~1.8e-2: too
close).

At fp8 PE speed the ReLU+bias PSUM->SBUF eviction is the wall: only
Act and DVE can read PSUM (GpSimd cannot, DMA cannot), and a single
[128,512] tile costs ~860ns on either engine under full overlap, ~60%
of it fixed per-instruction overhead + traversal at 1 elem/lane/cycle.
Mitigations:
  - super-groups: two row-groups share one [128,2,512] two-bank PSUM
    tile per h-slice m (written by two matmuls, same per-partition
    bias), relu'd with a single instruction: measured 1113ns (Act) /
    1283ns (DVE) per 2 tiles.  PSUM budget: 3 pair tiles rotating (6
    banks) + 1 shared out bank.
  - greedy Act/DVE assignment by measured busy-time (~54/46 split).
  - W2 matmuls are batched 5 super-groups at a time and gated on the
    last ReLU of their batch (a bypass-op data dependency through a
    copy of the W2 weights); without the gate the tile scheduler
    trickles W2s between W1 matmuls one at a time, paying the ~130ns
    quad<->mains array-drain transition on every matmul (~20us).
  - snake order over m between super-groups (one fewer LDWEIGHTS
    transition per boundary).
"""

import numpy as np

import concourse.bass as bass
import concourse.tile as tile
from concourse import bacc, mybir

N, D, H, C = 262144, 256, 512, 10
N_BAGS, BAG_SIZE = 4096, 64
N_CORES = 8
R = N // N_CORES            # rows per core
BPC = N_BAGS // N_CORES     # bags per core == free dim of each row-group
KT = D // 128               # contraction tiles (2)
MT = H // 128               # H tiles (4)

F32 = mybir.dt.float32
F16 = mybir.dt.float16
FP8 = mybir.dt.float8e4
AF = mybir.ActivationFunctionType
ALU = mybir.AluOpType
PM = mybir.MatmulPerfMode


def build(nc: bass.Bass, bag: int = BAG_SIZE, bpc: int = BPC):
    """Emit the per-core program.  bag = rows per bag (= number of
    row-groups), bpc = bags per core (= free dim, <= 512)."""
    xT = nc.declare_dram_parameter("xh", [128, bag // 2, KT, 2, bpc], FP8,
                                   isOutput=False)
    w1 = nc.declare_dram_parameter("w1h", [128, KT, H], FP8, isOutput=False)
    b1 = nc.declare_dram_parameter("b1h", [128, MT], F32, isOutput=False)
    w2 = nc.declare_dram_parameter("w2h", [128, MT, C], F16, isOutput=False)
    b2 = nc.declare_dram_parameter("b2h", [C, 1], F32, isOutput=False)
    out = nc.declare_dram_parameter("out", [C, bpc], F32, isOutput=True)

    with tile.TileContext(nc) as tc:
        with (
            tc.tile_pool(name="const", bufs=1) as cpool,
            tc.tile_pool(name="xin", bufs=8) as xpool,
            tc.tile_pool(name="hrelu", bufs=8) as hpool,
            tc.tile_pool(name="w2g", bufs=2) as gpool,
            tc.tile_pool(name="fin", bufs=1) as fpool,
            tc.tile_pool(name="ps_ht", bufs=3, space="PSUM") as pspool,  # 3x2 banks
            tc.tile_pool(name="ps_out", bufs=1, space="PSUM") as popool,
        ):
            # const loads go on the Activation HWDGE queue so the SP queue
            # starts issuing the (latency-critical) first x tiles at once
            w1_sb = cpool.tile([128, KT, H], FP8)
            for k in range(KT):
                nc.gpsimd.dma_start(out=w1_sb[:, k, :], in_=w1[:, k, :])
            b1_sb = cpool.tile([128, MT], F32)
            nc.gpsimd.dma_start(out=b1_sb[:], in_=b1[:])
            w2_sb = cpool.tile([128, MT, C], F16)
            nc.gpsimd.dma_start(out=w2_sb[:], in_=w2[:])
            b2_sb = cpool.tile([C, 1], F32)
            nc.gpsimd.dma_start(out=b2_sb[:], in_=b2[:])

            # bag-sum accumulators: col-group m holds partial (over H slice
            # m) of out.T at partitions [32m, 32m+10).  The has_written
            # clear of start=True is per-partition (HW-verified), so two
            # col-groups can share a bank on disjoint partitions: {0,2} in
            # bank A, {1,3} in bank B — freeing banks for ht pipelining.
            out_psb = popool.tile([128, bpc], F32, tag="outb",
                                  name="out_psb")
            out_ps = [out_psb for m in range(MT)]

            QB = 10

            def emit_w2(gprev, htrs_by_g, w2w):
                # htrs_by_g: {g: [4 APs]}
                for m in range(MT):
                    nc.tensor.matmul(
                        out_ps[m][32 * m:32 * m + C, :],
                        lhsT=w2w[:, m, :],
                        rhs=htrs_by_g[gprev][m],
                        start=(gprev == 0),
                        stop=(gprev == bag - 1),
                        tile_position=(0, 32 * m),
                        skip_group_check=True,
                    )

            # measured per-instruction busy (ns): singles and 2-group pairs
            ACT_1, DVE_1 = 687.0, 750.0
            ACT_2, DVE_2 = 1113.0, 1289.0
            eng_busy = {"a": 0.0, "d": 0.0}

            def emit_relu(htr_ap, ht_ap, m, cost_a, cost_d, accum=None):
                eng = ("a" if eng_busy["a"] + cost_a <= eng_busy["d"] + cost_d
                       else "d")
                if eng == "a":
                    eng_busy["a"] += cost_a
                    return nc.scalar.activation(
                        htr_ap, ht_ap, AF.Relu,
                        bias=b1_sb[:, m:m + 1], scale=1.0,
                        accum_out=accum,
                    )
                else:
                    eng_busy["d"] += cost_d
                    return nc.vector.tensor_scalar(
                        out=htr_ap, in0=ht_ap,
                        scalar1=b1_sb[:, m:m + 1], scalar2=0.0,
                        op0=ALU.add, op1=ALU.max,
                        accum_out=accum,
                    )

            pending = {}          # g -> [4 htr APs]
            pend_order = []

            # super-groups: two row-groups (2*sg, 2*sg+1) share one
            # [128, 1024] two-bank PSUM tile per m (written by two
            # matmuls) and ONE relu instruction (same per-partition bias),
            # amortizing the ~450ns fixed engine overhead over 2 tiles.
            for sg in range(bag // 2):
                xg = xpool.tile([128, KT, 2, bpc], FP8, tag="x",
                                name=f"x_{sg}")
                if sg == 0:
                    for j in range(2):
                        nc.sync.dma_start(
                            out=xg[:, :, j, :],
                            in_=xT[:, sg, :, j, :],
                        )
                else:
                    nc.sync.dma_start(out=xg[:], in_=xT[:, sg, :, :, :])

                htps = {}
                morder = (range(MT) if sg % 2 == 0
                          else range(MT - 1, -1, -1))
                for m in morder:
                    htp = pspool.tile([128, 2, bpc], F32, tag="htp",
                                      name=f"ht_{sg}_{m}")
                    for j in range(2):
                        nc.tensor.matmul(
                            htp[:, j, :],
                            lhsT=w1_sb[:, :, 128 * m:128 * (m + 1)],
                            rhs=xg[:, :, j, :],
                            start=True, stop=True,
                            perf_mode=PM.DoubleRow,
                        )
                    htrp = hpool.tile([128, 2, bpc], F16, tag=f"htr{m}",
                                      name=f"htr_{sg}_{m}")
                    emit_relu(htrp[:], htp[:], m, ACT_2, DVE_2)
                    htps[m] = htrp
                for j in range(2):
                    g = 2 * sg + j
                    pending[g] = [htps[m][:, j, :] for m in range(MT)]
                    pend_order.append(g)

                g = 2 * sg + 1
                # keep the newest super-group pending (its relu is still
                # in flight)
                if (sg % 5 == 3) or sg == 29 or sg == 30 or sg == bag // 2 - 1:
                    flush = (pend_order if sg == bag // 2 - 1
                             else pend_order[:-2])
                    keep = [] if sg == bag // 2 - 1 else pend_order[-2:]
                    # gate: a zero derived from the last flushed htr, added
                    # into a copy of the W2 weights.  Every W2 of the batch
                    # then becomes ready at once (bulk quad burst, one
                    # mains<->quad transition pair instead of ~2/group).
                    gsc = gpool.tile([128, 1], F32, tag="gsc",
                                     name=f"gsc_{sg}")
                    nc.vector.tensor_scalar(
                        out=gsc[:], in0=pending[flush[-1]][MT - 1][:, 0:1],
                        scalar1=0.0, scalar2=None, op0=ALU.mult,
                    )
                    w2w = gpool.tile([128, MT, C], F16, tag="w2gate",
                                     name=f"w2w_{sg}")
                    nc.vector.tensor_scalar(
                        out=w2w[:], in0=w2_sb[:],
                        scalar1=gsc[:], scalar2=None, op0=ALU.bypass,
                    )
                    for gp in flush:
                        emit_w2(gp, pending, w2w)
                    pending = {gq: pending[gq] for gq in keep}
                    pend_order = keep

            # combine the 4 partials + b2 (each op may read only one PSUM
            # operand; b2 fused into the first op)
            accb = fpool.tile([C, bpc], F32, tag="accb")
            nc.scalar.activation(
                accb[:], out_ps[2][64:64 + C, :], AF.Copy,
                bias=0.0, scale=1.0,
            )
            acc = fpool.tile([C, bpc], F32, tag="acc")
            nc.vector.tensor_scalar(
                out=acc[:], in0=out_ps[0][0:C, :], scalar1=b2_sb[:],
                scalar2=None, op0=ALU.add,
            )
            nc.vector.tensor_add(acc[:], acc[:], out_ps[1][32:32 + C, :])
            nc.vector.tensor_add(acc[:], acc[:], out_ps[3][96:96 + C, :])
            nc.vector.tensor_add(acc[:], acc[:], accb[:])
            nc.sync.dma_start(out=out[:], in_=acc[:])


def _np_fp8():
    import ml_dtypes
    return ml_dtypes.float8_e4m3


def host_prep_shared(W1, b1, W2, b2, bag=BAG_SIZE):
    w1h = np.ascontiguousarray(
        W1.reshape(KT, 128, H).transpose(1, 0, 2)).astype(_np_fp8())
    b1h = np.ascontiguousarray(
        b1.reshape(MT, 128).T).astype(np.float32)
    w2h = np.ascontiguousarray(
        (W2 / bag).reshape(MT, 128, C).transpose(1, 0, 2)).astype(np.float16)
    b2h = np.ascontiguousarray(b2.reshape(C, 1)).astype(np.float32)
    return {"w1h": w1h, "b1h": b1h, "w2h": w2h, "b2h": b2h}


def host_prep_x(xs, bag=BAG_SIZE):
    """xs: [r, D] rows of one core -> xh [128, bag//2, KT, 2, bpc]."""
    r = xs.shape[0]
    bpc = r // bag
    xh = xs.reshape(bpc, bag // 2, 2, KT, 128).transpose(4, 1, 3, 2, 0)
    return np.ascontiguousarray(xh).astype(_np_fp8())


_BUILT = None


def _get_built():
    global _BUILT
    if _BUILT is None:
        nc = bacc.Bacc("TRN2")
        build(nc)
        nc.compile()
        _BUILT = nc
    return _BUILT


def run(x, W1, b1, W2, b2, ids=None, trace=False):
    from concourse.bass_utils import run_bass_kernel_spmd

    nc = _get_built()
    shared = host_prep_shared(W1, b1, W2, b2)
    in_maps = []
    for c in range(N_CORES):
        xs = np.asarray(x[c * R:(c + 1) * R])
        in_maps.append({"xh": host_prep_x(xs), **shared})
    res = run_bass_kernel_spmd(
        nc, in_maps, core_ids=list(range(N_CORES)), trace=trace
    )
    outs = [res.results[c]["out"] for c in range(N_CORES)]
    full = np.concatenate([o.T for o in outs], axis=0).astype(np.float32)
    return full, res


def kernel(x, W1, b1, W2, b2, ids=None):
    full, _ = run(x, W1, b1, W2, b2, ids)
    return full


# revision 39
# speedup vs baseline: 1.0157x; 1.0157x over previous
"""BagModel (segment_reduce) Trainium2 kernel.

Computes out = (1/64 * segment_sum(relu(x @ W1 + b1))) @ W2 + b2 for
4096 bags of exactly 64 consecutive rows each, sharded bag-aligned
across 8 NeuronCores (512 bags / 32768 rows per core, weights
replicated, no cross-core communication).

Layout trick: the host permutes each core's x-shard so row-group g
contains row g of every bag, with the contraction dim D on partitions
(xh[p, sg, k, j, b] = x[b*64 + 2*sg + j, 128*k + p]).  The per-bag
segment-sum falls out of PSUM matmul accumulation: the second (W2)
matmul accumulates over the 64 row-groups with start/stop flags, so no
explicit reduction pass over h is ever needed.  The 4 H-slices of the
W2 matmul go to 4 distinct PE column-groups (tile_position) and
genuinely overlap in the array (~50ns/quad issue spacing inside a
batched burst); all four accumulate into ONE shared PSUM bank on
disjoint partitions (32m .. 32m+10).

fp8: x and W1 are quantized to float8e4 (e4m3) on the host and the W1
matmul runs in MatmulPerfMode.DoubleRow, packing the full D=256
contraction into one instruction per (group, h-slice) at 2x bf16
throughput (measured 213ns per [128,2,512]-in/[128,512]-out matmul).
End-to-end rel err vs the fp32 reference is 1.373e-2 (fp8 matmul
quantization dominates; verified identical in host simulation), within
the 2e-2 gate.  htr/W2 stay fp16 (fp8 htr would land at ~1.8e-2: too
close).

At fp8 PE speed the ReLU+bias PSUM->SBUF eviction is the wall: only
Act and DVE can read PSUM (GpSimd cannot, DMA cannot), and a single
[128,512] tile costs ~860ns on either engine under full overlap, ~60%
of it fixed per-instruction overhead + traversal at 1 elem/lane/cycle.
Mitigations:
  - super-groups: two row-groups share one [128,2,512] two-bank PSUM
    tile per h-slice m (written by two matmuls, same per-partition
    bias), relu'd with a single instruction: measured 1113ns (Act) /
    1283ns (DVE) per 2 tiles.  PSUM budget: 3 pair tiles rotating (6
    banks) + 1 shared out bank.
  - greedy Act/DVE assignment by measured busy-time (~54/46 split).
  - W2 matmuls are batched 5 super-groups at a time and gated on the
    last ReLU of their batch (a bypass-op data dependency through a
    copy of the W2 weights); without the gate the tile scheduler
    trickles W2s between W1 matmuls one at a time, paying the ~130ns
    quad<->mains array-drain transition on every matmul (~20us).
  - snake order over m between super-groups (one fewer LDWEIGHTS
    transition per boundary).
"""

import numpy as np

import concourse.bass as bass
import concourse.tile as tile
from concourse import bacc, mybir

N, D, H, C = 262144, 256, 512, 10
N_BAGS, BAG_SIZE = 4096, 64
N_CORES = 8
R = N // N_CORES            # rows per core
BPC = N_BAGS // N_CORES     # bags per core == free dim of each row-group
KT = D // 128               # contraction tiles (2)
MT = H // 128               # H tiles (4)

F32 = mybir.dt.float32
F16 = mybir.dt.float16
FP8 = mybir.dt.float8e4
AF = mybir.ActivationFunctionType
ALU = mybir.AluOpType
PM = mybir.MatmulPerfMode


def build(nc: bass.Bass, bag: int = BAG_SIZE, bpc: int = BPC):
    """Emit the per-core program.  bag = rows per bag (= number of
    row-groups), bpc = bags per core (= free dim, <= 512)."""
    xT = nc.declare_dram_parameter("xh", [128, bag // 2, KT, 2, bpc], FP8,
                                   isOutput=False)
    w1 = nc.declare_dram_parameter("w1h", [128, KT, H], FP8, isOutput=False)
    b1 = nc.declare_dram_parameter("b1h", [128, MT], F32, isOutput=False)
    w2 = nc.declare_dram_parameter("w2h", [128, MT, C], F16, isOutput=False)
    b2 = nc.declare_dram_parameter("b2h", [C, 1], F32, isOutput=False)
    out = nc.declare_dram_parameter("out", [C, bpc], F32, isOutput=True)

    with tile.TileContext(nc) as tc:
        with (
            tc.tile_pool(name="const", bufs=1) as cpool,
            tc.tile_pool(name="xin", bufs=8) as xpool,
            tc.tile_pool(name="hrelu", bufs=12) as hpool,
            tc.tile_pool(name="w2g", bufs=2) as gpool,
            tc.tile_pool(name="fin", bufs=1) as fpool,
            tc.tile_pool(name="ps_ht", bufs=3, space="PSUM") as pspool,  # 3x2 banks
            tc.tile_pool(name="ps_out", bufs=1, space="PSUM") as popool,
        ):
            # const loads go on the Activation HWDGE queue so the SP queue
            # starts issuing the (latency-critical) first x tiles at once
            w1_sb = cpool.tile([128, KT, H], FP8)
            for k in range(KT):
                nc.gpsimd.dma_start(out=w1_sb[:, k, :], in_=w1[:, k, :])
            b1_sb = cpool.tile([128, MT], F32)
            nc.gpsimd.dma_start(out=b1_sb[:], in_=b1[:])
            w2_sb = cpool.tile([128, MT, C], F16)
            nc.gpsimd.dma_start(out=w2_sb[:], in_=w2[:])
            b2_sb = cpool.tile([C, 1], F32)
            nc.gpsimd.dma_start(out=b2_sb[:], in_=b2[:])

            # bag-sum accumulators: col-group m holds partial (over H slice
            # m) of out.T at partitions [32m, 32m+10).  The has_written
            # clear of start=True is per-partition (HW-verified), so two
            # col-groups can share a bank on disjoint partitions: {0,2} in
            # bank A, {1,3} in bank B — freeing banks for ht pipelining.
            out_psb = popool.tile([128, bpc], F32, tag="outb",
                                  name="out_psb")
            out_ps = [out_psb for m in range(MT)]

            QB = 10

            def emit_w2(gprev, htrs_by_g, w2w):
                # htrs_by_g: {g: [4 APs]}
                for m in range(MT):
                    nc.tensor.matmul(
                        out_ps[m][32 * m:32 * m + C, :],
                        lhsT=w2w[:, m, :],
                        rhs=htrs_by_g[gprev][m],
                        start=(gprev == 0),
                        stop=(gprev == bag - 1),
                        tile_position=(0, 32 * m),
                        skip_group_check=True,
                    )

            # measured per-instruction busy (ns): singles and 2-group pairs
            ACT_1, DVE_1 = 687.0, 750.0
            ACT_2, DVE_2 = 1113.0, 1289.0
            eng_busy = {"a": 0.0, "d": 0.0}

            def emit_relu(htr_ap, ht_ap, m, cost_a, cost_d, accum=None):
                eng = ("a" if eng_busy["a"] + cost_a <= eng_busy["d"] + cost_d
                       else "d")
                if eng == "a":
                    eng_busy["a"] += cost_a
                    return nc.scalar.activation(
                        htr_ap, ht_ap, AF.Relu,
                        bias=b1_sb[:, m:m + 1], scale=1.0,
                        accum_out=accum,
                    )
                else:
                    eng_busy["d"] += cost_d
                    return nc.vector.tensor_scalar(
                        out=htr_ap, in0=ht_ap,
                        scalar1=b1_sb[:, m:m + 1], scalar2=0.0,
                        op0=ALU.add, op1=ALU.max,
                        accum_out=accum,
                    )

            pending = {}          # g -> [4 htr APs]
            pend_order = []

            # super-groups: two row-groups (2*sg, 2*sg+1) share one
            # [128, 1024] two-bank PSUM tile per m (written by two
            # matmuls) and ONE relu instruction (same per-partition bias),
            # amortizing the ~450ns fixed engine overhead over 2 tiles.
            for sg in range(bag // 2):
                xg = xpool.tile([128, KT, 2, bpc], FP8, tag="x",
                                name=f"x_{sg}")
                if sg == 0:
                    for j in range(2):
                        nc.sync.dma_start(
                            out=xg[:, :, j, :],
                            in_=xT[:, sg, :, j, :],
                        )
                else:
                    nc.sync.dma_start(out=xg[:], in_=xT[:, sg, :, :, :])

                htps = {}
                morder = (range(MT) if sg % 2 == 0
                          else range(MT - 1, -1, -1))
                for m in morder:
                    htp = pspool.tile([128, 2, bpc], F32, tag="htp",
                                      name=f"ht_{sg}_{m}")
                    for j in range(2):
                        nc.tensor.matmul(
                            htp[:, j, :],
                            lhsT=w1_sb[:, :, 128 * m:128 * (m + 1)],
                            rhs=xg[:, :, j, :],
                            start=True, stop=True,
                            perf_mode=PM.DoubleRow,
                        )
                    htrp = hpool.tile([128, 2, bpc], F16, tag=f"htr{m}",
                                      name=f"htr_{sg}_{m}")
                    emit_relu(htrp[:], htp[:], m, ACT_2, DVE_2)
                    htps[m] = htrp
                for j in range(2):
                    g = 2 * sg + j
                    pending[g] = [htps[m][:, j, :] for m in range(MT)]
                    pend_order.append(g)

                g = 2 * sg + 1
                # keep the newest super-group pending (its relu is still
                # in flight)
                if (sg % 5 == 3) or sg == 29 or sg == 30 or sg == bag // 2 - 1:
                    flush = (pend_order if sg == bag // 2 - 1
                             else pend_order[:-2])
                    keep = [] if sg == bag // 2 - 1 else pend_order[-2:]
                    # gate: a zero derived from the last flushed htr, added
                    # into a copy of the W2 weights.  Every W2 of the batch
                    # then becomes ready at once (bulk quad burst, one
                    # mains<->quad transition pair instead of ~2/group).
                    gsc = gpool.tile([128, 1], F32, tag="gsc",
                                     name=f"gsc_{sg}")
                    nc.vector.tensor_scalar(
                        out=gsc[:], in0=pending[flush[-1]][MT - 1][:, 0:1],
                        scalar1=0.0, scalar2=None, op0=ALU.mult,
                    )
                    w2w = gpool.tile([128, MT, C], F16, tag="w2gate",
                                     name=f"w2w_{sg}")
                    nc.vector.tensor_scalar(
                        out=w2w[:], in0=w2_sb[:],
                        scalar1=gsc[:], scalar2=None, op0=ALU.bypass,
                    )
                    for gp in flush:
                        emit_w2(gp, pending, w2w)
                    pending = {gq: pending[gq] for gq in keep}
                    pend_order = keep

            # combine the 4 partials + b2 (each op may read only one PSUM
            # operand; b2 fused into the first op)
            accb = fpool.tile([C, bpc], F32, tag="accb")
            nc.scalar.activation(
                accb[:], out_ps[2][64:64 + C, :], AF.Copy,
                bias=0.0, scale=1.0,
            )
            acc = fpool.tile([C, bpc], F32, tag="acc")
            nc.vector.tensor_scalar(
                out=acc[:], in0=out_ps[0][0:C, :], scalar1=b2_sb[:],
                scalar2=None, op0=ALU.add,
            )
            nc.vector.tensor_add(acc[:], acc[:], out_ps[1][32:32 + C, :])
            nc.vector.tensor_add(acc[:], acc[:], out_ps[3][96:96 + C, :])
            nc.vector.tensor_add(acc[:], acc[:], accb[:])
            nc.sync.dma_start(out=out[:], in_=acc[:])


def _np_fp8():
    import ml_dtypes
    return ml_dtypes.float8_e4m3


def host_prep_shared(W1, b1, W2, b2, bag=BAG_SIZE):
    w1h = np.ascontiguousarray(
        W1.reshape(KT, 128, H).transpose(1, 0, 2)).astype(_np_fp8())
    b1h = np.ascontiguousarray(
        b1.reshape(MT, 128).T).astype(np.float32)
    w2h = np.ascontiguousarray(
        (W2 / bag).reshape(MT, 128, C).transpose(1, 0, 2)).astype(np.float16)
    b2h = np.ascontiguousarray(b2.reshape(C, 1)).astype(np.float32)
    return {"w1h": w1h, "b1h": b1h, "w2h": w2h, "b2h": b2h}


def host_prep_x(xs, bag=BAG_SIZE):
    """xs: [r, D] rows of one core -> xh [128, bag//2, KT, 2, bpc]."""
    r = xs.shape[0]
    bpc = r // bag
    xh = xs.reshape(bpc, bag // 2, 2, KT, 128).transpose(4, 1, 3, 2, 0)
    return np.ascontiguousarray(xh).astype(_np_fp8())


_BUILT = None


def _get_built():
    global _BUILT
    if _BUILT is None:
        nc = bacc.Bacc("TRN2")
        build(nc)
        nc.compile()
        _BUILT = nc
    return _BUILT


def run(x, W1, b1, W2, b2, ids=None, trace=False):
    from concourse.bass_utils import run_bass_kernel_spmd

    nc = _get_built()
    shared = host_prep_shared(W1, b1, W2, b2)
    in_maps = []
    for c in range(N_CORES):
        xs = np.asarray(x[c * R:(c + 1) * R])
        in_maps.append({"xh": host_prep_x(xs), **shared})
    res = run_bass_kernel_spmd(
        nc, in_maps, core_ids=list(range(N_CORES)), trace=trace
    )
    outs = [res.results[c]["out"] for c in range(N_CORES)]
    full = np.concatenate([o.T for o in outs], axis=0).astype(np.float32)
    return full, res


def kernel(x, W1, b1, W2, b2, ids=None):
    full, _ = run(x, W1, b1, W2, b2, ids)
    return full


# revision 40
# speedup vs baseline: 1.0233x; 1.0075x over previous
"""BagModel (segment_reduce) Trainium2 kernel.

Computes out = (1/64 * segment_sum(relu(x @ W1 + b1))) @ W2 + b2 for
4096 bags of exactly 64 consecutive rows each, sharded bag-aligned
across 8 NeuronCores (512 bags / 32768 rows per core, weights
replicated, no cross-core communication).

Layout trick: the host permutes each core's x-shard so row-group g
contains row g of every bag, with the contraction dim D on partitions
(xh[p, sg, k, j, b] = x[b*64 + 2*sg + j, 128*k + p]).  The per-bag
segment-sum falls out of PSUM matmul accumulation: the second (W2)
matmul accumulates over the 64 row-groups with start/stop flags, so no
explicit reduction pass over h is ever needed.  The 4 H-slices of the
W2 matmul go to 4 distinct PE column-groups (tile_position) and
genuinely overlap in the array (~50ns/quad issue spacing inside a
batched burst); all four accumulate into ONE shared PSUM bank on
disjoint partitions (32m .. 32m+10).

fp8: x and W1 are quantized to float8e4 (e4m3) on the host and the W1
matmul runs in MatmulPerfMode.DoubleRow, packing the full D=256
contraction into one instruction per (group, h-slice) at 2x bf16
throughput (measured 213ns per [128,2,512]-in/[128,512]-out matmul).
End-to-end rel err vs the fp32 reference is 1.373e-2 (fp8 matmul
quantization dominates; verified identical in host simulation), within
the 2e-2 gate.  htr/W2 stay fp16 (fp8 htr would land at ~1.8e-2: too
close).

At fp8 PE speed the ReLU+bias PSUM->SBUF eviction is the wall: only
Act and DVE can read PSUM (GpSimd cannot, DMA cannot), and a single
[128,512] tile costs ~860ns on either engine under full overlap, ~60%
of it fixed per-instruction overhead + traversal at 1 elem/lane/cycle.
Mitigations:
  - super-groups: two row-groups share one [128,2,512] two-bank PSUM
    tile per h-slice m (written by two matmuls, same per-partition
    bias), relu'd with a single instruction: measured 1113ns (Act) /
    1283ns (DVE) per 2 tiles.  PSUM budget: 3 pair tiles rotating (6
    banks) + 1 shared out bank.
  - greedy Act/DVE assignment by measured busy-time (~54/46 split).
  - W2 matmuls are batched 5 super-groups at a time and gated on the
    last ReLU of their batch (a bypass-op data dependency through a
    copy of the W2 weights); without the gate the tile scheduler
    trickles W2s between W1 matmuls one at a time, paying the ~130ns
    quad<->mains array-drain transition on every matmul (~20us).
  - snake order over m between super-groups (one fewer LDWEIGHTS
    transition per boundary).
"""

import numpy as np

import concourse.bass as bass
import concourse.tile as tile
from concourse import bacc, mybir

N, D, H, C = 262144, 256, 512, 10
N_BAGS, BAG_SIZE = 4096, 64
N_CORES = 8
R = N // N_CORES            # rows per core
BPC = N_BAGS // N_CORES     # bags per core == free dim of each row-group
KT = D // 128               # contraction tiles (2)
MT = H // 128               # H tiles (4)

F32 = mybir.dt.float32
F16 = mybir.dt.float16
FP8 = mybir.dt.float8e4
AF = mybir.ActivationFunctionType
ALU = mybir.AluOpType
PM = mybir.MatmulPerfMode


def build(nc: bass.Bass, bag: int = BAG_SIZE, bpc: int = BPC):
    """Emit the per-core program.  bag = rows per bag (= number of
    row-groups), bpc = bags per core (= free dim, <= 512)."""
    xT = nc.declare_dram_parameter("xh", [128, bag // 2, KT, 2, bpc], FP8,
                                   isOutput=False)
    w1 = nc.declare_dram_parameter("w1h", [128, KT, H], FP8, isOutput=False)
    b1 = nc.declare_dram_parameter("b1h", [128, MT], F32, isOutput=False)
    w2 = nc.declare_dram_parameter("w2h", [128, MT, C], F16, isOutput=False)
    b2 = nc.declare_dram_parameter("b2h", [C, 1], F32, isOutput=False)
    out = nc.declare_dram_parameter("out", [C, bpc], F32, isOutput=True)

    with tile.TileContext(nc) as tc:
        with (
            tc.tile_pool(name="const", bufs=1) as cpool,
            tc.tile_pool(name="xin", bufs=8) as xpool,
            tc.tile_pool(name="hrelu", bufs=12) as hpool,
            tc.tile_pool(name="w2g", bufs=2) as gpool,
            tc.tile_pool(name="fin", bufs=1) as fpool,
            tc.tile_pool(name="ps_ht", bufs=3, space="PSUM") as pspool,  # 3x2 banks
            tc.tile_pool(name="ps_out", bufs=1, space="PSUM") as popool,
        ):
            # const loads go on the Activation HWDGE queue so the SP queue
            # starts issuing the (latency-critical) first x tiles at once
            w1_sb = cpool.tile([128, KT, H], FP8)
            for k in range(KT):
                nc.gpsimd.dma_start(out=w1_sb[:, k, :], in_=w1[:, k, :])
            b1_sb = cpool.tile([128, MT], F32)
            nc.gpsimd.dma_start(out=b1_sb[:], in_=b1[:])
            w2_sb = cpool.tile([128, MT, C], F16)
            nc.gpsimd.dma_start(out=w2_sb[:], in_=w2[:])
            b2_sb = cpool.tile([C, 1], F32)
            nc.gpsimd.dma_start(out=b2_sb[:], in_=b2[:])

            # bag-sum accumulators: col-group m holds partial (over H slice
            # m) of out.T at partitions [32m, 32m+10).  The has_written
            # clear of start=True is per-partition (HW-verified), so all
            # four col-groups share ONE bank on disjoint partitions,
            # freeing banks for ht pipelining.
            out_psb = popool.tile([128, bpc], F32, tag="outb",
                                  name="out_psb")
            out_ps = [out_psb for m in range(MT)]

            QB = 10

            def emit_w2(gprev, htrs_by_g, w2w):
                # htrs_by_g: {g: [4 APs]}
                for m in range(MT):
                    nc.tensor.matmul(
                        out_ps[m][32 * m:32 * m + C, :],
                        lhsT=w2w[:, m, :],
                        rhs=htrs_by_g[gprev][m],
                        start=(gprev == 0),
                        stop=(gprev == bag - 1),
                        tile_position=(0, 32 * m),
                        skip_group_check=True,
                    )

            # measured per-instruction busy (ns): singles and 2-group pairs
            ACT_1, DVE_1 = 687.0, 750.0
            ACT_2, DVE_2 = 1113.0, 1289.0
            eng_busy = {"a": 0.0, "d": 0.0}

            def emit_relu(htr_ap, ht_ap, m, cost_a, cost_d, accum=None):
                eng = ("a" if eng_busy["a"] + cost_a <= eng_busy["d"] + cost_d
                       else "d")
                if eng == "a":
                    eng_busy["a"] += cost_a
                    return nc.scalar.activation(
                        htr_ap, ht_ap, AF.Relu,
                        bias=b1_sb[:, m:m + 1], scale=1.0,
                        accum_out=accum,
                    )
                else:
                    eng_busy["d"] += cost_d
                    return nc.vector.tensor_scalar(
                        out=htr_ap, in0=ht_ap,
                        scalar1=b1_sb[:, m:m + 1], scalar2=0.0,
                        op0=ALU.add, op1=ALU.max,
                        accum_out=accum,
                    )

            pending = {}          # g -> [4 htr APs]
            pend_order = []

            # super-groups: two row-groups (2*sg, 2*sg+1) share one
            # [128, 1024] two-bank PSUM tile per m (written by two
            # matmuls) and ONE relu instruction (same per-partition bias),
            # amortizing the ~450ns fixed engine overhead over 2 tiles.
            for sg in range(bag // 2):
                xg = xpool.tile([128, KT, 2, bpc], FP8, tag="x",
                                name=f"x_{sg}")
                if sg == 0:
                    for j in range(2):
                        nc.sync.dma_start(
                            out=xg[:, :, j, :],
                            in_=xT[:, sg, :, j, :],
                        )
                else:
                    nc.sync.dma_start(out=xg[:], in_=xT[:, sg, :, :, :])

                htps = {}
                morder = (range(MT) if sg % 2 == 0
                          else range(MT - 1, -1, -1))
                for m in morder:
                    htp = pspool.tile([128, 2, bpc], F32, tag="htp",
                                      name=f"ht_{sg}_{m}")
                    for j in range(2):
                        nc.tensor.matmul(
                            htp[:, j, :],
                            lhsT=w1_sb[:, :, 128 * m:128 * (m + 1)],
                            rhs=xg[:, :, j, :],
                            start=True, stop=True,
                            perf_mode=PM.DoubleRow,
                        )
                    htrp = hpool.tile([128, 2, bpc], F16, tag=f"htr{m}",
                                      name=f"htr_{sg}_{m}")
                    emit_relu(htrp[:], htp[:], m, ACT_2, DVE_2)
                    htps[m] = htrp
                for j in range(2):
                    g = 2 * sg + j
                    pending[g] = [htps[m][:, j, :] for m in range(MT)]
                    pend_order.append(g)

                g = 2 * sg + 1
                # keep the newest super-group pending (its relu is still
                # in flight)
                if (sg % 5 == 3) or sg == 29 or sg == 30 or sg == bag // 2 - 1:
                    flush = (pend_order if sg == bag // 2 - 1
                             else pend_order[:-2])
                    keep = [] if sg == bag // 2 - 1 else pend_order[-2:]
                    # gate: a zero derived from the last flushed htr, added
                    # into a copy of the W2 weights.  Every W2 of the batch
                    # then becomes ready at once (bulk quad burst, one
                    # mains<->quad transition pair instead of ~2/group).
                    gsc = gpool.tile([128, 1], F32, tag="gsc",
                                     name=f"gsc_{sg}")
                    nc.vector.tensor_scalar(
                        out=gsc[:], in0=pending[flush[-1]][MT - 1][:, 0:1],
                        scalar1=0.0, scalar2=None, op0=ALU.mult,
                    )
                    w2w = gpool.tile([128, MT, C], F16, tag="w2gate",
                                     name=f"w2w_{sg}")
                    nc.vector.tensor_scalar(
                        out=w2w[:], in0=w2_sb[:],
                        scalar1=gsc[:], scalar2=None, op0=ALU.bypass,
                    )
                    for gp in flush:
                        emit_w2(gp, pending, w2w)
                    pending = {gq: pending[gq] for gq in keep}
                    pend_order = keep

            # combine the 4 partials + b2 (each op may read only one PSUM
            # operand; b2 fused into the first op)
            accb = fpool.tile([C, bpc], F32, tag="accb")
            nc.scalar.activation(
                accb[:], out_ps[2][64:64 + C, :], AF.Copy,
                bias=0.0, scale=1.0,
            )
            acc = fpool.tile([C, bpc], F32, tag="acc")
            nc.vector.tensor_scalar(
                out=acc[:], in0=out_ps[0][0:C, :], scalar1=b2_sb[:],
                scalar2=None, op0=ALU.add,
            )
            nc.vector.tensor_add(acc[:], acc[:], out_ps[1][32:32 + C, :])
            nc.vector.tensor_add(acc[:], acc[:], out_ps[3][96:96 + C, :])
            nc.vector.tensor_add(acc[:], acc[:], accb[:])
            nc.sync.dma_start(out=out[:], in_=acc[:])


def _np_fp8():
    import ml_dtypes
    return ml_dtypes.float8_e4m3


def host_prep_shared(W1, b1, W2, b2, bag=BAG_SIZE):
    w1h = np.ascontiguousarray(
        W1.reshape(KT, 128, H).transpose(1, 0, 2)).astype(_np_fp8())
    b1h = np.ascontiguousarray(
        b1.reshape(MT, 128).T).astype(np.float32)
    w2h = np.ascontiguousarray(
        (W2 / bag).reshape(MT, 128, C).transpose(1, 0, 2)).astype(np.float16)
    b2h = np.ascontiguousarray(b2.reshape(C, 1)).astype(np.float32)
    return {"w1h": w1h, "b1h": b1h, "w2h": w2h, "b2h": b2h}


def host_prep_x(xs, bag=BAG_SIZE):
    """xs: [r, D] rows of one core -> xh [128, bag//2, KT, 2, bpc]."""
    r = xs.shape[0]
    bpc = r // bag
    xh = xs.reshape(bpc, bag // 2, 2, KT, 128).transpose(4, 1, 3, 2, 0)
    return np.ascontiguousarray(xh).astype(_np_fp8())


_BUILT = None


def _get_built():
    global _BUILT
    if _BUILT is None:
        nc = bacc.Bacc("TRN2")
        build(nc)
        nc.compile()
        _BUILT = nc
    return _BUILT


def run(x, W1, b1, W2, b2, ids=None, trace=False):
    from concourse.bass_utils import run_bass_kernel_spmd

    nc = _get_built()
    shared = host_prep_shared(W1, b1, W2, b2)
    in_maps = []
    for c in range(N_CORES):
        xs = np.asarray(x[c * R:(c + 1) * R])
        in_maps.append({"xh": host_prep_x(xs), **shared})
    res = run_bass_kernel_spmd(
        nc, in_maps, core_ids=list(range(N_CORES)), trace=trace
    )
    outs = [res.results[c]["out"] for c in range(N_CORES)]
    full = np.concatenate([o.T for o in outs], axis=0).astype(np.float32)
    return full, res


def kernel(x, W1, b1, W2, b2, ids=None):
    full, _ = run(x, W1, b1, W2, b2, ids)
    return full
